# revision 22
# baseline (speedup 1.0000x reference)
"""Causal self-attention (B=2, T=2048, d_model=1024, H=16) on 8 TRN2 NeuronCores.

Sharding: core c handles batch b = c//4 and head group g = c%4 (heads 4g..4g+3).
Each core computes QKV projection for its heads, causal attention, and a partial
output projection y_partial = attn_out @ Wo[g*256:(g+1)*256, :]. The host sums
the 4 partials per batch (the tensor-parallel all-reduce, done on host) after
upcasting the bf16 partials.

Schedule per rep: att(ic=0) for both head pairs -> QKV projection of the second
token half -> att(ic=1) -> output projection -> next rep's first-half QKV.
Attention interleaves the two heads of a pair at j-block granularity so PE runs
~2 pipeline steps ahead of each exp, hiding the ACT handoff latency; the
projection blocks between attention sections give ACT time to drain its exp
backlog (they have no ACT work - all PSUM->SBUF copies are on DVE).

All matmul operands are bf16 (host converts); accumulation stays fp32 in PSUM.
PSUM: tag "s" 2x[128,1024] shared by S-tiles/projections, tag "o" 2 slots for
PV accumulators (one per in-flight head) shared with the V-projection.
"""
import sys

sys.path.insert(0, "/opt/trn_rl_repo")

import numpy as np

B, T, C = 2, 2048, 1024
NH_TOT = 16
HD = 64
NH = 4          # heads per core
CO = NH * HD    # 256 channels per core
NCORES = 8
SCALE = 1.0 / 32.0  # d_model ** -0.5

_compiled = None


def _build(nrep=1, trace_sim=False):
    import concourse.bass as bass  # noqa: F401
    import concourse.mybir as mybir
    import concourse.tile as tile
    from concourse import bacc

    F32 = mybir.dt.float32
    BF16 = mybir.dt.bfloat16
    MULT = mybir.AluOpType.mult
    EXP = mybir.ActivationFunctionType.Exp

    nc = bacc.Bacc("TRN2", target_bir_lowering=False)

    xT = nc.declare_dram_parameter("xT", [C, T], BF16, isOutput=False)
    wq = nc.declare_dram_parameter("wq", [C, CO], BF16, isOutput=False)
    wk = nc.declare_dram_parameter("wk", [C, CO], BF16, isOutput=False)
    wv = nc.declare_dram_parameter("wv", [C, CO], BF16, isOutput=False)
    wo = nc.declare_dram_parameter("wo", [CO, C], BF16, isOutput=False)
    mask = nc.declare_dram_parameter("mask", [128, 128], BF16, isOutput=False)
    y = nc.declare_dram_parameter("y", [T, C], BF16, isOutput=True)

    xT_t = xT.rearrange("(o p) t -> p o t", p=128)   # [128, 8, 2048]
    wq_t = wq.rearrange("(o p) m -> p o m", p=128)   # [128, 8, 256]
    wk_t = wk.rearrange("(o p) m -> p o m", p=128)
    wv_t = wv.rearrange("(o p) m -> p o m", p=128)
    wo_t = wo.rearrange("(o p) m -> p o m", p=128)   # [128, 2, 1024]

    with tile.TileContext(nc, trace_sim=trace_sim) as tc:
        with (
            nc.allow_low_precision(reason="bf16 matmul pipeline"),
            tc.tile_pool(name="wpool", bufs=1) as wpool,
            tc.tile_pool(name="xpool", bufs=2) as xpool,
            tc.tile_pool(name="qkvpool", bufs=2) as qkvpool,
            tc.tile_pool(name="otpool", bufs=2) as otpool,
            tc.tile_pool(name="etpool", bufs=10) as etpool,
            tc.tile_pool(name="rppool", bufs=3) as rppool,
            tc.tile_pool(name="bcpool", bufs=3) as bcpool,
            tc.tile_pool(name="ypool", bufs=3) as ypool,
            tc.tile_pool(name="psum", bufs=2, space="PSUM") as psum,
        ):
            wq_sb = wpool.tile([128, 8, CO], BF16, tag="wq")
            wk_sb = wpool.tile([128, 8, CO], BF16, tag="wk")
            wv_sb = wpool.tile([128, 8, CO], BF16, tag="wv")
            wo_sb = wpool.tile([128, 2, C], BF16, tag="wo")
            mask_sb = wpool.tile([128, 128], BF16, tag="mask")
            nc.sync.dma_start(wq_sb[:], wq_t[:])
            nc.sync.dma_start(wk_sb[:], wk_t[:])
            nc.sync.dma_start(wv_sb[:], wv_t[:])
            nc.sync.dma_start(wo_sb[:], wo_t[:])
            nc.sync.dma_start(mask_sb[:], mask[:])

            reps = {}

            class Rep:
                def __init__(self, r):
                    self.r = r
                    self.x = xpool.tile([128, 8, T], BF16, tag="xT", name=f"x{r}")
                    self.qT = qkvpool.tile([128, 2, T], BF16, tag="qT", name=f"q{r}")
                    self.kT = qkvpool.tile([128, 2, T], BF16, tag="kT", name=f"k{r}")
                    # V' per (t-block, head): 64 cols of V then a ones column
                    self.vp = qkvpool.tile([128, 16, NH, HD + 1], BF16, tag="vp",
                                           name=f"v{r}")
                    self.oT = otpool.tile([128, 2, T], BF16, tag="oT", name=f"o{r}")
                    nc.vector.memset(self.vp[:, :, :, HD], 1.0)

            def get_rep(r):
                if r not in reps:
                    reps[r] = Rep(r)
                return reps[r]

            def emit_xdma(r):
                rep = get_rep(r)
                for t8 in range(2):
                    for kc in range(8):
                        sl = slice(t8 * 1024, (t8 + 1) * 1024)
                        nc.sync.dma_start(rep.x[:, kc, sl], xT_t[:, kc, sl])

            def phase1_block(r, t8):
                """QKV projection for one 1024-token half (solid PE block)."""
                rep = get_rep(r)
                for m in range(2):
                    for w_sb, dst in ((wk_sb, rep.kT), (wq_sb, rep.qT)):
                        for half in range(2):
                            pq = psum.tile([128, 1024], F32, tag="s",
                                           name="pq")[:, 0:512]
                            t0c = t8 * 1024 + half * 512
                            for kc in range(8):
                                nc.tensor.matmul(
                                    pq[:],
                                    w_sb[:, kc, m * 128:(m + 1) * 128],
                                    rep.x[:, kc, t0c:t0c + 512],
                                    start=(kc == 0),
                                    stop=(kc == 7),
                                )
                            nc.vector.tensor_copy(dst[:, m, t0c:t0c + 512], pq[:])
                for tb in range(8 * t8, 8 * t8 + 8):
                    pv = psum.tile([128, 1024], F32, tag="o",
                                   name="pv")[:, 0:CO]
                    for kc in range(8):
                        nc.tensor.matmul(
                            pv[:],
                            rep.x[:, kc, tb * 128:(tb + 1) * 128],
                            wv_sb[:, kc, :],
                            start=(kc == 0),
                            stop=(kc == 7),
                        )
                    nc.vector.tensor_copy(
                        rep.vp[:, tb, :, 0:HD],
                        pv[:].rearrange("p (h d) -> p h d", h=NH),
                    )

            def att_pair(r, pair, ic):
                """Causal attention for head pair, query block ic (1024 wide).

                The two heads are interleaved at j-block granularity: PE is
                two pipeline steps ahead of the exp it waits on.
                """
                rep = get_rep(r)
                heads = (2 * pair, 2 * pair + 1)
                i_base = 1024 * ic
                jb_last = 8 * ic + 7
                pos = {
                    h: psum.tile([65, 1024], F32, tag="o",
                                 name=f"po{r}_{h}_{ic}")
                    for h in heads
                }

                def emit_s(h, jb):
                    po2, mo2 = h % 2, h // 2
                    k_h = rep.kT[64 * po2:64 * po2 + 64, mo2, :]
                    q_h = rep.qT[64 * po2:64 * po2 + 64, mo2, :]
                    i0 = max(i_base, 128 * jb)
                    ps_s = psum.tile([128, 1024], F32, tag="s", name="ps_s")
                    off = i0 - i_base
                    while off < 1024:
                        w = min(512 - off % 512, 1024 - off)
                        nc.tensor.matmul(
                            ps_s[:, off:off + w],
                            k_h[:, jb * 128:(jb + 1) * 128],
                            q_h[:, i_base + off:i_base + off + w],
                            start=True,
                            stop=True,
                        )
                        off += w
                    et = etpool.tile([128, 1024], BF16, tag="et", name="et")
                    o0 = i0 - i_base
                    nc.scalar.activation(
                        et[:, o0:1024], ps_s[:, o0:1024], EXP, scale=SCALE,
                    )
                    if 128 * jb >= i_base:
                        nc.vector.tensor_tensor(
                            et[:, o0:o0 + 128], et[:, o0:o0 + 128],
                            mask_sb[:], MULT,
                        )
                    return et, i0

                def emit_pv(h, jb, et, i0):
                    # PSUM accumulation groups are bank-granular (2KB): close
                    # each bank's group on the last jb whose causal range
                    # still touches that bank.  The diagonal 128-strip depends
                    # on the DVE mask multiply - emit it LAST so the wide
                    # chunks (which only need the exp) keep PE busy while the
                    # mask handoff completes.
                    o0 = i0 - i_base
                    diag = 128 * jb >= i_base
                    chunks = []
                    off = o0 + 128 if diag else o0
                    while off < 1024:
                        w = min(512 - off % 512, 1024 - off)
                        chunks.append((off, w))
                        off += w
                    if diag:
                        chunks.append((o0, min(128, 1024 - o0)))
                    first_idx, last_idx = {}, {}
                    for idx, (off, w) in enumerate(chunks):
                        bank = off // 512
                        if bank not in first_idx:
                            first_idx[bank] = idx
                        last_idx[bank] = idx
                    for idx, (off, w) in enumerate(chunks):
                        bank = off // 512
                        jb_stop = min(jb_last,
                                      (i_base + 512 * (bank + 1) - 1) // 128)
                        nc.tensor.matmul(
                            pos[h][:, off:off + w],
                            rep.vp[:, jb, h, :],
                            et[:, off:off + w],
                            start=(jb == 0 and first_idx[bank] == idx),
                            stop=(jb == jb_stop and last_idx[bank] == idx),
                        )

                pend = [emit_s(h, 0) for h in heads]
                for jb in range(jb_last + 1):
                    nxt = None
                    if jb < jb_last:
                        nxt = [emit_s(h, jb + 1) for h in heads]
                    for hi, h in enumerate(heads):
                        emit_pv(h, jb, *pend[hi])
                    if nxt is not None:
                        pend = nxt

                # normalize: recip of sums row (PSUM p64 -> SBUF p64), DMA
                # broadcast across partitions, then fold the PSUM->SBUF copy
                # into the multiply.
                isl = slice(i_base, i_base + 1024)
                for h in heads:
                    po2, mo2 = h % 2, h // 2
                    rp = rppool.tile([65, 1024], F32, tag="rp", name="rp")
                    nc.vector.reciprocal(rp[64:65, :], pos[h][64:65, :])
                    bc = bcpool.tile([64, 1024], F32, tag="bc", name="bc")
                    nc.sync.dma_start(
                        bc[:], rp[64:65, None, :].to_broadcast([1, 64, 1024])
                    )
                    nc.vector.tensor_tensor(
                        rep.oT[64 * po2:64 * po2 + 64, mo2, isl],
                        pos[h][0:64, :], bc[:], MULT,
                    )

            def outproj_block(r, ic):
                """Output projection for one 1024-token half."""
                rep = get_rep(r)
                for tb2 in range(4 * ic, 4 * ic + 4):
                    y2 = ypool.tile([128, 2, C], BF16, tag="yt", name="y2")
                    for sub in range(2):
                        tb = 2 * tb2 + sub
                        for nk in range(2):
                            py = psum.tile([128, 1024], F32, tag="s",
                                           name="py")[:, 0:512]
                            for cp in range(2):
                                nc.tensor.matmul(
                                    py[:],
                                    rep.oT[:, cp, tb * 128:(tb + 1) * 128],
                                    wo_sb[:, cp, nk * 512:(nk + 1) * 512],
                                    start=(cp == 0),
                                    stop=(cp == 1),
                                )
                            nc.vector.tensor_copy(
                                y2[:, sub, nk * 512:(nk + 1) * 512], py[:])
                    nc.sync.dma_start(
                        y[tb2 * 256:(tb2 + 1) * 256, :].rearrange(
                            "(b p) c -> p b c", p=128
                        ),
                        y2[:],
                    )

            # ---------------- emission ----------------
            emit_xdma(0)
            phase1_block(0, t8=0)
            for r in range(nrep):
                if r + 1 < nrep:
                    emit_xdma(r + 1)
                att_pair(r, 0, ic=0)
                att_pair(r, 1, ic=0)
                phase1_block(r, t8=1)   # ACT drains ic0 exp backlog here
                att_pair(r, 0, ic=1)
                att_pair(r, 1, ic=1)
                outproj_block(r, ic=0)
                outproj_block(r, ic=1)  # ACT drains ic1 exp backlog here
                if r + 1 < nrep:
                    phase1_block(r + 1, t8=0)

    nc.compile()
    return nc


def _get_nc():
    global _compiled
    if _compiled is None:
        _compiled = _build()
    return _compiled


class _Runner:
    """Compiled PJRT executor for the SPMD kernel, reusable across calls."""

    def __init__(self, nc):
        import jax
        import concourse.mybir as mybir
        from concourse import bass2jax
        from jax.experimental.shard_map import shard_map
        from jax.sharding import Mesh, PartitionSpec

        self.jax = jax
        self.nc = nc
        bass2jax.install_neuronx_cc_hook()

        partition_name = (
            nc.partition_id_tensor.name if nc.partition_id_tensor else None
        )
        in_names, out_names, out_avals, zero_outs = [], [], [], []
        for alloc in nc.m.functions[0].allocations:
            if not isinstance(alloc, mybir.MemoryLocationSet):
                continue
            name = alloc.memorylocations[0].name
            if alloc.kind == "ExternalInput":
                if name != partition_name:
                    in_names.append(name)
            elif alloc.kind == "ExternalOutput":
                out_names.append(name)
                shape = tuple(alloc.tensor_shape)
                dtype = mybir.dt.np(alloc.dtype)
                out_avals.append(jax.core.ShapedArray(shape, dtype))
                zero_outs.append(np.zeros(shape, dtype))
        self.in_names = in_names
        self.out_names = out_names
        self.out_avals = out_avals
        self.zero_outs = zero_outs
        all_names = tuple(in_names + out_names)

        if partition_name is not None:
            all_names = all_names + (partition_name,)

        def _body(*args):
            operands = list(args)
            if partition_name is not None:
                operands.append(bass2jax.partition_id_tensor())
            outs = bass2jax._bass_exec_p.bind(
                *operands,
                out_avals=tuple(out_avals),
                in_names=all_names,
                out_names=tuple(out_names),
                lowering_input_output_aliases=(),
                sim_require_finite=True,
                sim_require_nnan=True,
                nc=nc,
            )
            return tuple(outs)

        devices = jax.devices()[:NCORES]
        assert len(devices) == NCORES
        mesh = Mesh(np.asarray(devices), ("core",))
        self._sharding = jax.sharding.NamedSharding(mesh, PartitionSpec("core"))
        n_args = len(in_names) + len(out_names)
        self.fn = jax.jit(
            shard_map(
                _body,
                mesh=mesh,
                in_specs=(PartitionSpec("core"),) * n_args,
                out_specs=(PartitionSpec("core"),) * len(out_names),
                check_rep=False,
            ),
            keep_unused=True,
        )

    def device_args(self, in_maps):
        args = [
            np.concatenate([np.asarray(m[name]) for m in in_maps], axis=0)
            for name in self.in_names
        ]
        args += [
            np.zeros((NCORES * z.shape[0], *z.shape[1:]), z.dtype)
            for z in self.zero_outs
        ]
        return [self.jax.device_put(a, self._sharding) for a in args]

    def run_device(self, dev_args):
        return self.fn(*dev_args)

    def run(self, in_maps):
        out_arrs = self.fn(*self.device_args(in_maps))
        return [
            {
                name: np.asarray(out_arrs[i]).reshape(
                    NCORES, *self.out_avals[i].shape
                )[c]
                for i, name in enumerate(self.out_names)
            }
            for c in range(NCORES)
        ]


_runner = None


def _get_runner():
    global _runner
    if _runner is None:
        _runner = _Runner(_get_nc())
    return _runner


def make_in_maps(x, Wqkv, Wo):
    import ml_dtypes

    bf16 = ml_dtypes.bfloat16
    x = np.asarray(x, dtype=np.float32)
    Wqkv = np.asarray(Wqkv, dtype=np.float32)
    Wo = np.asarray(Wo, dtype=np.float32)
    mask = np.triu(np.ones((128, 128), dtype=np.float32)).astype(bf16)
    in_maps = []
    for c in range(NCORES):
        b, g = c // 4, c % 4
        in_maps.append({
            "xT": np.ascontiguousarray(x[b].T).astype(bf16),
            "wq": np.ascontiguousarray(
                Wqkv[:, g * CO:(g + 1) * CO]).astype(bf16),
            "wk": np.ascontiguousarray(
                Wqkv[:, C + g * CO:C + (g + 1) * CO]).astype(bf16),
            "wv": np.ascontiguousarray(
                Wqkv[:, 2 * C + g * CO:2 * C + (g + 1) * CO]).astype(bf16),
            "wo": np.ascontiguousarray(Wo[g * CO:(g + 1) * CO, :]).astype(bf16),
            "mask": mask,
        })
    return in_maps


def gather_output(results):
    y = np.zeros((B, T, C), dtype=np.float32)
    for c in range(NCORES):
        y[c // 4] += results[c]["y"].astype(np.float32)
    return y


def kernel(x, Wqkv, Wo):
    runner = _get_runner()
    in_maps = make_in_maps(x, Wqkv, Wo)
    return gather_output(runner.run(in_maps))


# revision 23
# speedup vs baseline: 1.0554x; 1.0554x over previous
"""Causal self-attention (B=2, T=2048, d_model=1024, H=16) on 8 TRN2 NeuronCores.

Sharding: core c handles batch b = c//4 and head group g = c%4 (heads 4g..4g+3).
Each core computes QKV projection for its heads, causal attention, and a partial
output projection y_partial = attn_out @ Wo[g*256:(g+1)*256, :]. The host sums
the 4 partials per batch (the tensor-parallel all-reduce, done on host) after
upcasting the bf16 partials.

Schedule per rep: att(ic=0) for both head pairs -> QKV projection of the second
token half -> att(ic=1) -> output projection -> next rep's first-half QKV.
Attention interleaves the two heads of a pair at j-block granularity so PE runs
~2 pipeline steps ahead of each exp, hiding the ACT handoff latency; the
projection blocks between attention sections give ACT time to drain its exp
backlog (they have no ACT work - all PSUM->SBUF copies are on DVE).

All matmul operands are bf16 (host converts); accumulation stays fp32 in PSUM.
PSUM: tag "s" 2x[128,1024] shared by S-tiles/projections, tag "o" 2 slots for
PV accumulators (one per in-flight head) shared with the V-projection.
"""
import sys

sys.path.insert(0, "/opt/trn_rl_repo")

import numpy as np

B, T, C = 2, 2048, 1024
NH_TOT = 16
HD = 64
NH = 4          # heads per core
CO = NH * HD    # 256 channels per core
NCORES = 8
SCALE = 1.0 / 32.0  # d_model ** -0.5

_compiled = None


def _build(nrep=1, trace_sim=False):
    import concourse.bass as bass  # noqa: F401
    import concourse.mybir as mybir
    import concourse.tile as tile
    from concourse import bacc

    F32 = mybir.dt.float32
    BF16 = mybir.dt.bfloat16
    MULT = mybir.AluOpType.mult
    EXP = mybir.ActivationFunctionType.Exp

    nc = bacc.Bacc("TRN2", target_bir_lowering=False)

    xT = nc.declare_dram_parameter("xT", [C, T], BF16, isOutput=False)
    wq = nc.declare_dram_parameter("wq", [C, CO], BF16, isOutput=False)
    wk = nc.declare_dram_parameter("wk", [C, CO], BF16, isOutput=False)
    wv = nc.declare_dram_parameter("wv", [C, CO], BF16, isOutput=False)
    wo = nc.declare_dram_parameter("wo", [CO, C], BF16, isOutput=False)
    mask = nc.declare_dram_parameter("mask", [128, 128], BF16, isOutput=False)
    y = nc.declare_dram_parameter("y", [T, C], BF16, isOutput=True)

    xT_t = xT.rearrange("(o p) t -> p o t", p=128)   # [128, 8, 2048]
    wq_t = wq.rearrange("(o p) m -> p o m", p=128)   # [128, 8, 256]
    wk_t = wk.rearrange("(o p) m -> p o m", p=128)
    wv_t = wv.rearrange("(o p) m -> p o m", p=128)
    wo_t = wo.rearrange("(o p) m -> p o m", p=128)   # [128, 2, 1024]

    with tile.TileContext(nc, trace_sim=trace_sim) as tc:
        with (
            nc.allow_low_precision(reason="bf16 matmul pipeline"),
            tc.tile_pool(name="wpool", bufs=1) as wpool,
            tc.tile_pool(name="xpool", bufs=2) as xpool,
            tc.tile_pool(name="qkvpool", bufs=2) as qkvpool,
            tc.tile_pool(name="otpool", bufs=2) as otpool,
            tc.tile_pool(name="etpool", bufs=6) as etpool,
            tc.tile_pool(name="rppool", bufs=2) as rppool,
            tc.tile_pool(name="bcpool", bufs=2) as bcpool,
            tc.tile_pool(name="ypool", bufs=2) as ypool,
            tc.tile_pool(name="psum", bufs=2, space="PSUM") as psum,
        ):
            wq_sb = wpool.tile([128, 8, CO], BF16, tag="wq")
            wk_sb = wpool.tile([128, 8, CO], BF16, tag="wk")
            wv_sb = wpool.tile([128, 8, CO], BF16, tag="wv")
            wo_sb = wpool.tile([128, 2, C], BF16, tag="wo")
            mask_sb = wpool.tile([128, 128], BF16, tag="mask")
            nc.sync.dma_start(wq_sb[:], wq_t[:])
            nc.sync.dma_start(wk_sb[:], wk_t[:])
            nc.sync.dma_start(wv_sb[:], wv_t[:])
            nc.sync.dma_start(wo_sb[:], wo_t[:])
            nc.sync.dma_start(mask_sb[:], mask[:])

            reps = {}

            class Rep:
                def __init__(self, r):
                    self.r = r
                    self.x = xpool.tile([128, 8, T], BF16, tag="xT", name=f"x{r}")
                    self.qT = qkvpool.tile([128, 2, T], BF16, tag="qT", name=f"q{r}")
                    self.kT = qkvpool.tile([128, 2, T], BF16, tag="kT", name=f"k{r}")
                    # V' per (t-block, head): 64 cols of V then a ones column
                    self.vp = qkvpool.tile([128, 16, NH, HD + 1], BF16, tag="vp",
                                           name=f"v{r}")
                    self.oT = otpool.tile([128, 2, T], BF16, tag="oT", name=f"o{r}")
                    nc.vector.memset(self.vp[:, :, :, HD], 1.0)

            def get_rep(r):
                if r not in reps:
                    reps[r] = Rep(r)
                return reps[r]

            def emit_xdma(r):
                rep = get_rep(r)
                for t8 in range(2):
                    for kc in range(8):
                        sl = slice(t8 * 1024, (t8 + 1) * 1024)
                        nc.sync.dma_start(rep.x[:, kc, sl], xT_t[:, kc, sl])

            def phase1_block(r, t8):
                """QKV projection for one 1024-token half (solid PE block)."""
                rep = get_rep(r)
                for m in range(2):
                    for w_sb, dst in ((wk_sb, rep.kT), (wq_sb, rep.qT)):
                        for half in range(2):
                            pq = psum.tile([128, 1024], F32, tag="s",
                                           name="pq")[:, 0:512]
                            t0c = t8 * 1024 + half * 512
                            for kc in range(8):
                                nc.tensor.matmul(
                                    pq[:],
                                    w_sb[:, kc, m * 128:(m + 1) * 128],
                                    rep.x[:, kc, t0c:t0c + 512],
                                    start=(kc == 0),
                                    stop=(kc == 7),
                                )
                            nc.vector.tensor_copy(dst[:, m, t0c:t0c + 512], pq[:])
                for tb in range(8 * t8, 8 * t8 + 8):
                    pv = psum.tile([128, 1024], F32, tag="o",
                                   name="pv")[:, 0:CO]
                    for kc in range(8):
                        nc.tensor.matmul(
                            pv[:],
                            rep.x[:, kc, tb * 128:(tb + 1) * 128],
                            wv_sb[:, kc, :],
                            start=(kc == 0),
                            stop=(kc == 7),
                        )
                    nc.vector.tensor_copy(
                        rep.vp[:, tb, :, 0:HD],
                        pv[:].rearrange("p (h d) -> p h d", h=NH),
                    )

            def att_pair(r, pair, ic):
                """Causal attention for head pair, query block ic (1024 wide).

                The two heads are interleaved at j-block granularity: PE is
                two pipeline steps ahead of the exp it waits on.
                """
                rep = get_rep(r)
                heads = (2 * pair, 2 * pair + 1)
                i_base = 1024 * ic
                jb_last = 8 * ic + 7
                pos = {
                    h: psum.tile([65, 1024], F32, tag="o",
                                 name=f"po{r}_{h}_{ic}")
                    for h in heads
                }

                def emit_s(h, jb):
                    po2, mo2 = h % 2, h // 2
                    k_h = rep.kT[64 * po2:64 * po2 + 64, mo2, :]
                    q_h = rep.qT[64 * po2:64 * po2 + 64, mo2, :]
                    i0 = max(i_base, 128 * jb)
                    ps_s = psum.tile([128, 1024], F32, tag="s", name="ps_s")
                    off = i0 - i_base
                    while off < 1024:
                        w = min(512 - off % 512, 1024 - off)
                        nc.tensor.matmul(
                            ps_s[:, off:off + w],
                            k_h[:, jb * 128:(jb + 1) * 128],
                            q_h[:, i_base + off:i_base + off + w],
                            start=True,
                            stop=True,
                        )
                        off += w
                    et = etpool.tile([128, 1024], BF16, tag="et", name="et")
                    o0 = i0 - i_base
                    nc.scalar.activation(
                        et[:, o0:1024], ps_s[:, o0:1024], EXP, scale=SCALE,
                    )
                    if 128 * jb >= i_base:
                        nc.vector.tensor_tensor(
                            et[:, o0:o0 + 128], et[:, o0:o0 + 128],
                            mask_sb[:], MULT,
                        )
                    return et, i0

                def emit_pv(h, jb, et, i0):
                    # PSUM accumulation groups are bank-granular (2KB): close
                    # each bank's group on the last jb whose causal range
                    # still touches that bank.  The diagonal 128-strip depends
                    # on the DVE mask multiply - emit it LAST so the wide
                    # chunks (which only need the exp) keep PE busy while the
                    # mask handoff completes.
                    o0 = i0 - i_base
                    diag = 128 * jb >= i_base
                    chunks = []
                    off = o0 + 128 if diag else o0
                    while off < 1024:
                        w = min(512 - off % 512, 1024 - off)
                        chunks.append((off, w))
                        off += w
                    if diag:
                        chunks.append((o0, min(128, 1024 - o0)))
                    first_idx, last_idx = {}, {}
                    for idx, (off, w) in enumerate(chunks):
                        bank = off // 512
                        if bank not in first_idx:
                            first_idx[bank] = idx
                        last_idx[bank] = idx
                    for idx, (off, w) in enumerate(chunks):
                        bank = off // 512
                        jb_stop = min(jb_last,
                                      (i_base + 512 * (bank + 1) - 1) // 128)
                        nc.tensor.matmul(
                            pos[h][:, off:off + w],
                            rep.vp[:, jb, h, :],
                            et[:, off:off + w],
                            start=(jb == 0 and first_idx[bank] == idx),
                            stop=(jb == jb_stop and last_idx[bank] == idx),
                        )

                pend = [emit_s(h, 0) for h in heads]
                for jb in range(jb_last + 1):
                    nxt = None
                    if jb < jb_last:
                        nxt = [emit_s(h, jb + 1) for h in heads]
                    for hi, h in enumerate(heads):
                        emit_pv(h, jb, *pend[hi])
                    if nxt is not None:
                        pend = nxt

                # normalize: recip of sums row (PSUM p64 -> SBUF p64), DMA
                # broadcast across partitions, then fold the PSUM->SBUF copy
                # into the multiply.
                isl = slice(i_base, i_base + 1024)
                for h in heads:
                    po2, mo2 = h % 2, h // 2
                    rp = rppool.tile([65, 1024], F32, tag="rp", name="rp")
                    nc.vector.reciprocal(rp[64:65, :], pos[h][64:65, :])
                    bc = bcpool.tile([64, 1024], F32, tag="bc", name="bc")
                    nc.sync.dma_start(
                        bc[:], rp[64:65, None, :].to_broadcast([1, 64, 1024])
                    )
                    nc.vector.tensor_tensor(
                        rep.oT[64 * po2:64 * po2 + 64, mo2, isl],
                        pos[h][0:64, :], bc[:], MULT,
                    )

            def outproj_block(r, ic):
                """Output projection for one 1024-token half."""
                rep = get_rep(r)
                for tb2 in range(4 * ic, 4 * ic + 4):
                    y2 = ypool.tile([128, 2, C], BF16, tag="yt", name="y2")
                    for sub in range(2):
                        tb = 2 * tb2 + sub
                        for nk in range(2):
                            py = psum.tile([128, 1024], F32, tag="s",
                                           name="py")[:, 0:512]
                            for cp in range(2):
                                nc.tensor.matmul(
                                    py[:],
                                    rep.oT[:, cp, tb * 128:(tb + 1) * 128],
                                    wo_sb[:, cp, nk * 512:(nk + 1) * 512],
                                    start=(cp == 0),
                                    stop=(cp == 1),
                                )
                            nc.vector.tensor_copy(
                                y2[:, sub, nk * 512:(nk + 1) * 512], py[:])
                    nc.sync.dma_start(
                        y[tb2 * 256:(tb2 + 1) * 256, :].rearrange(
                            "(b p) c -> p b c", p=128
                        ),
                        y2[:],
                    )

            # ---------------- emission ----------------
            emit_xdma(0)
            phase1_block(0, t8=0)
            for r in range(nrep):
                if r + 1 < nrep:
                    emit_xdma(r + 1)
                att_pair(r, 0, ic=0)
                att_pair(r, 1, ic=0)
                phase1_block(r, t8=1)   # ACT drains ic0 exp backlog here
                att_pair(r, 0, ic=1)
                att_pair(r, 1, ic=1)
                outproj_block(r, ic=0)
                outproj_block(r, ic=1)  # ACT drains ic1 exp backlog here
                if r + 1 < nrep:
                    phase1_block(r + 1, t8=0)

    nc.compile()
    return nc


def _get_nc():
    global _compiled
    if _compiled is None:
        _compiled = _build()
    return _compiled


class _Runner:
    """Compiled PJRT executor for the SPMD kernel, reusable across calls."""

    def __init__(self, nc):
        import jax
        import concourse.mybir as mybir
        from concourse import bass2jax
        from jax.experimental.shard_map import shard_map
        from jax.sharding import Mesh, PartitionSpec

        self.jax = jax
        self.nc = nc
        bass2jax.install_neuronx_cc_hook()

        partition_name = (
            nc.partition_id_tensor.name if nc.partition_id_tensor else None
        )
        in_names, out_names, out_avals, zero_outs = [], [], [], []
        for alloc in nc.m.functions[0].allocations:
            if not isinstance(alloc, mybir.MemoryLocationSet):
                continue
            name = alloc.memorylocations[0].name
            if alloc.kind == "ExternalInput":
                if name != partition_name:
                    in_names.append(name)
            elif alloc.kind == "ExternalOutput":
                out_names.append(name)
                shape = tuple(alloc.tensor_shape)
                dtype = mybir.dt.np(alloc.dtype)
                out_avals.append(jax.core.ShapedArray(shape, dtype))
                zero_outs.append(np.zeros(shape, dtype))
        self.in_names = in_names
        self.out_names = out_names
        self.out_avals = out_avals
        self.zero_outs = zero_outs
        all_names = tuple(in_names + out_names)

        if partition_name is not None:
            all_names = all_names + (partition_name,)

        def _body(*args):
            operands = list(args)
            if partition_name is not None:
                operands.append(bass2jax.partition_id_tensor())
            outs = bass2jax._bass_exec_p.bind(
                *operands,
                out_avals=tuple(out_avals),
                in_names=all_names,
                out_names=tuple(out_names),
                lowering_input_output_aliases=(),
                sim_require_finite=True,
                sim_require_nnan=True,
                nc=nc,
            )
            return tuple(outs)

        devices = jax.devices()[:NCORES]
        assert len(devices) == NCORES
        mesh = Mesh(np.asarray(devices), ("core",))
        self._sharding = jax.sharding.NamedSharding(mesh, PartitionSpec("core"))
        n_args = len(in_names) + len(out_names)
        self.fn = jax.jit(
            shard_map(
                _body,
                mesh=mesh,
                in_specs=(PartitionSpec("core"),) * n_args,
                out_specs=(PartitionSpec("core"),) * len(out_names),
                check_rep=False,
            ),
            keep_unused=True,
        )

    def device_args(self, in_maps):
        args = [
            np.concatenate([np.asarray(m[name]) for m in in_maps], axis=0)
            for name in self.in_names
        ]
        args += [
            np.zeros((NCORES * z.shape[0], *z.shape[1:]), z.dtype)
            for z in self.zero_outs
        ]
        return [self.jax.device_put(a, self._sharding) for a in args]

    def run_device(self, dev_args):
        return self.fn(*dev_args)

    def run(self, in_maps):
        out_arrs = self.fn(*self.device_args(in_maps))
        return [
            {
                name: np.asarray(out_arrs[i]).reshape(
                    NCORES, *self.out_avals[i].shape
                )[c]
                for i, name in enumerate(self.out_names)
            }
            for c in range(NCORES)
        ]


_runner = None


def _get_runner():
    global _runner
    if _runner is None:
        _runner = _Runner(_get_nc())
    return _runner


def make_in_maps(x, Wqkv, Wo):
    import ml_dtypes

    bf16 = ml_dtypes.bfloat16
    x = np.asarray(x, dtype=np.float32)
    Wqkv = np.asarray(Wqkv, dtype=np.float32)
    Wo = np.asarray(Wo, dtype=np.float32)
    mask = np.triu(np.ones((128, 128), dtype=np.float32)).astype(bf16)
    in_maps = []
    for c in range(NCORES):
        b, g = c // 4, c % 4
        in_maps.append({
            "xT": np.ascontiguousarray(x[b].T).astype(bf16),
            "wq": np.ascontiguousarray(
                Wqkv[:, g * CO:(g + 1) * CO]).astype(bf16),
            "wk": np.ascontiguousarray(
                Wqkv[:, C + g * CO:C + (g + 1) * CO]).astype(bf16),
            "wv": np.ascontiguousarray(
                Wqkv[:, 2 * C + g * CO:2 * C + (g + 1) * CO]).astype(bf16),
            "wo": np.ascontiguousarray(Wo[g * CO:(g + 1) * CO, :]).astype(bf16),
            "mask": mask,
        })
    return in_maps


def gather_output(results):
    y = np.zeros((B, T, C), dtype=np.float32)
    for c in range(NCORES):
        y[c // 4] += results[c]["y"].astype(np.float32)
    return y


def kernel(x, Wqkv, Wo):
    runner = _get_runner()
    in_maps = make_in_maps(x, Wqkv, Wo)
    return gather_output(runner.run(in_maps))


# revision 28
# speedup vs baseline: 1.1468x; 1.0866x over previous
"""Causal self-attention (B=2, T=2048, d_model=1024, H=16) on 8 TRN2 NeuronCores.

Sharding: core c handles batch b = c//4 and head group g = c%4 (heads 4g..4g+3).
Each core computes QKV projection for its heads, causal attention, and a partial
output projection y_partial = attn_out @ Wo[g*256:(g+1)*256, :]. The host sums
the 4 partials per batch (the tensor-parallel all-reduce, done on host) after
upcasting the bf16 partials.

Schedule per rep: att(ic=0) for both head pairs -> QKV projection of the second
token half -> att(ic=1) -> output projection -> next rep's first-half QKV.
Attention interleaves the two heads of a pair at j-block granularity so PE runs
~2 pipeline steps ahead of each exp, hiding the ACT handoff latency; the
projection blocks between attention sections give ACT time to drain its exp
backlog (they have no ACT work - all PSUM->SBUF copies are on DVE).

All matmul operands are bf16 (host converts); accumulation stays fp32 in PSUM.
PSUM: tag "s" 2x[128,1024] shared by S-tiles/projections, tag "o" 2 slots for
PV accumulators (one per in-flight head) shared with the V-projection.
"""
import sys

sys.path.insert(0, "/opt/trn_rl_repo")

import numpy as np

B, T, C = 2, 2048, 1024
NH_TOT = 16
HD = 64
NH = 4          # heads per core
CO = NH * HD    # 256 channels per core
NCORES = 8
SCALE = 1.0 / 32.0  # d_model ** -0.5

# attention schedule: "pair" = two heads interleaved, 2 S-slots;
# "single3" = one head per window, 3 S-slots (deeper S->exp WAR runahead)
ATT_MODE = "pair"

_compiled = None


def _build(nrep=1, trace_sim=False, att_mode=None, probe=None):
    import concourse.bass as bass  # noqa: F401
    import concourse.mybir as mybir
    import concourse.tile as tile
    from concourse import bacc

    F32 = mybir.dt.float32
    BF16 = mybir.dt.bfloat16
    MULT = mybir.AluOpType.mult
    EXP = mybir.ActivationFunctionType.Exp

    if att_mode is None:
        att_mode = ATT_MODE
    single3 = att_mode == "single3"
    s_kw = {"bufs": 3} if single3 else {}

    nc = bacc.Bacc("TRN2", target_bir_lowering=False)

    xT = nc.declare_dram_parameter("xT", [C, T], BF16, isOutput=False)
    wq = nc.declare_dram_parameter("wq", [C, CO], BF16, isOutput=False)
    wk = nc.declare_dram_parameter("wk", [C, CO], BF16, isOutput=False)
    wv = nc.declare_dram_parameter("wv", [C, CO], BF16, isOutput=False)
    wo = nc.declare_dram_parameter("wo", [CO, C], BF16, isOutput=False)
    mask = nc.declare_dram_parameter("mask", [128, 128], BF16, isOutput=False)
    y = nc.declare_dram_parameter("y", [T, C], BF16, isOutput=True)

    xT_t = xT.rearrange("(o p) t -> p o t", p=128)   # [128, 8, 2048]
    wq_t = wq.rearrange("(o p) m -> p o m", p=128)   # [128, 8, 256]
    wk_t = wk.rearrange("(o p) m -> p o m", p=128)
    wv_t = wv.rearrange("(o p) m -> p o m", p=128)
    wo_t = wo.rearrange("(o p) m -> p o m", p=128)   # [128, 2, 1024]

    with tile.TileContext(nc, trace_sim=trace_sim) as tc:
        with (
            nc.allow_low_precision(reason="bf16 matmul pipeline"),
            tc.tile_pool(name="wpool", bufs=1) as wpool,
            tc.tile_pool(name="xpool", bufs=2) as xpool,
            tc.tile_pool(name="qkvpool", bufs=2) as qkvpool,
            tc.tile_pool(name="otpool", bufs=2) as otpool,
            tc.tile_pool(name="etpool", bufs=6) as etpool,
            tc.tile_pool(name="rppool", bufs=2) as rppool,
            tc.tile_pool(name="bcpool", bufs=2) as bcpool,
            tc.tile_pool(name="ypool", bufs=2) as ypool,
            tc.tile_pool(name="psum", bufs=2, space="PSUM") as psum,
        ):
            wq_sb = wpool.tile([128, 8, CO], BF16, tag="wq")
            wk_sb = wpool.tile([128, 8, CO], BF16, tag="wk")
            wv_sb = wpool.tile([128, 8, CO], BF16, tag="wv")
            wo_sb = wpool.tile([128, 2, C], BF16, tag="wo")
            mask_sb = wpool.tile([128, 128], BF16, tag="mask")
            ones_sb = wpool.tile([1, 64], BF16, tag="ones")
            nc.vector.memset(ones_sb[:], 1.0)
            nc.sync.dma_start(wq_sb[:], wq_t[:])
            nc.sync.dma_start(wk_sb[:], wk_t[:])
            nc.sync.dma_start(wv_sb[:], wv_t[:])
            nc.sync.dma_start(wo_sb[:], wo_t[:])
            nc.sync.dma_start(mask_sb[:], mask[:])

            reps = {}

            class Rep:
                def __init__(self, r):
                    self.r = r
                    self.x = xpool.tile([128, 8, T], BF16, tag="xT", name=f"x{r}")
                    self.qT = qkvpool.tile([128, 2, T], BF16, tag="qT", name=f"q{r}")
                    self.kT = qkvpool.tile([128, 2, T], BF16, tag="kT", name=f"k{r}")
                    # V' per (t-block, head): 64 cols of V then a ones column
                    self.vp = qkvpool.tile([128, 16, NH, HD + 1], BF16, tag="vp",
                                           name=f"v{r}")
                    self.oT = otpool.tile([128, 2, T], BF16, tag="oT", name=f"o{r}")
                    nc.vector.memset(self.vp[:, :, :, HD], 1.0)

            def get_rep(r):
                if r not in reps:
                    reps[r] = Rep(r)
                return reps[r]

            def emit_xdma(r):
                rep = get_rep(r)
                for t8 in range(2):
                    for kc in range(8):
                        sl = slice(t8 * 1024, (t8 + 1) * 1024)
                        nc.sync.dma_start(rep.x[:, kc, sl], xT_t[:, kc, sl])

            def phase1_block(r, t8):
                """QKV projection for one 1024-token half (solid PE block)."""
                rep = get_rep(r)
                for m in range(2):
                    for w_sb, dst in ((wk_sb, rep.kT), (wq_sb, rep.qT)):
                        for half in range(2):
                            pq = psum.tile([128, 1024], F32, tag="s",
                                           name="pq", **s_kw)[:, 0:512]
                            t0c = t8 * 1024 + half * 512
                            for kc in range(8):
                                nc.tensor.matmul(
                                    pq[:],
                                    w_sb[:, kc, m * 128:(m + 1) * 128],
                                    rep.x[:, kc, t0c:t0c + 512],
                                    start=(kc == 0),
                                    stop=(kc == 7),
                                )
                            nc.vector.tensor_copy(dst[:, m, t0c:t0c + 512], pq[:])
                for tb in range(8 * t8, 8 * t8 + 8):
                    pv = psum.tile([128, 1024], F32, tag="s" if single3 else "o",
                                   name="pv", **s_kw)[:, 0:CO]
                    for kc in range(8):
                        nc.tensor.matmul(
                            pv[:],
                            rep.x[:, kc, tb * 128:(tb + 1) * 128],
                            wv_sb[:, kc, :],
                            start=(kc == 0),
                            stop=(kc == 7),
                        )
                    nc.vector.tensor_copy(
                        rep.vp[:, tb, :, 0:HD],
                        pv[:].rearrange("p (h d) -> p h d", h=NH),
                    )

            def att_group(r, heads, ic):
                """Causal attention for `heads`, query block ic (1024 wide).

                pair mode: two heads interleaved at j-block granularity.
                single3 mode: one head, S emitted two j-blocks ahead through
                three S-slots, widening the S->exp WAR runahead.
                """
                rep = get_rep(r)
                i_base = 1024 * ic
                jb_last = 8 * ic + 7
                pos = {
                    h: psum.tile([65, 1024], F32, tag="o",
                                 name=f"po{r}_{h}_{ic}",
                                 **({"bufs": 1} if single3 else {}))
                    for h in heads
                }

                def emit_s(h, jb):
                    po2, mo2 = h % 2, h // 2
                    k_h = rep.kT[64 * po2:64 * po2 + 64, mo2, :]
                    q_h = rep.qT[64 * po2:64 * po2 + 64, mo2, :]
                    i0 = max(i_base, 128 * jb)
                    ps_s = psum.tile([128, 1024], F32, tag="s", name="ps_s",
                                     **s_kw)
                    off = i0 - i_base
                    while off < 1024:
                        w = min(512 - off % 512, 1024 - off)
                        nc.tensor.matmul(
                            ps_s[:, off:off + w],
                            k_h[:, jb * 128:(jb + 1) * 128],
                            q_h[:, i_base + off:i_base + off + w],
                            start=True,
                            stop=True,
                        )
                        off += w
                    et = etpool.tile([128, 1024], BF16, tag="et", name="et")
                    o0 = i0 - i_base
                    nc.scalar.activation(
                        et[:, o0:1024], ps_s[:, o0:1024], EXP, scale=SCALE,
                    )
                    if probe == "2exp":
                        et2 = etpool.tile([128, 1024], BF16, tag="et2",
                                          name="et2")
                        nc.scalar.activation(
                            et2[:, o0:1024], ps_s[:, o0:1024], EXP, scale=SCALE,
                        )
                    if 128 * jb >= i_base:
                        nc.vector.tensor_tensor(
                            et[:, o0:o0 + 128], et[:, o0:o0 + 128],
                            mask_sb[:], MULT,
                        )
                    return et, i0

                def emit_pv(h, jb, et, i0):
                    # PSUM accumulation groups are bank-granular (2KB): close
                    # each bank's group on the last jb whose causal range
                    # still touches that bank.  The diagonal 128-strip depends
                    # on the DVE mask multiply - emit it LAST so the wide
                    # chunks (which only need the exp) keep PE busy while the
                    # mask handoff completes.
                    o0 = i0 - i_base
                    diag = 128 * jb >= i_base
                    chunks = []
                    off = o0 + 128 if diag else o0
                    while off < 1024:
                        w = min(512 - off % 512, 1024 - off)
                        chunks.append((off, w))
                        off += w
                    if diag:
                        chunks.append((o0, min(128, 1024 - o0)))
                    first_idx, last_idx = {}, {}
                    for idx, (off, w) in enumerate(chunks):
                        bank = off // 512
                        if bank not in first_idx:
                            first_idx[bank] = idx
                        last_idx[bank] = idx
                    for idx, (off, w) in enumerate(chunks):
                        bank = off // 512
                        jb_stop = min(jb_last,
                                      (i_base + 512 * (bank + 1) - 1) // 128)
                        nc.tensor.matmul(
                            pos[h][:, off:off + w],
                            rep.vp[:, jb, h, :],
                            et[:, off:off + w],
                            start=(jb == 0 and first_idx[bank] == idx),
                            stop=(jb == jb_stop and last_idx[bank] == idx),
                        )

                if single3:
                    h = heads[0]
                    pend = [emit_s(h, 0)]
                    if jb_last >= 1:
                        pend.append(emit_s(h, 1))
                    for jb in range(jb_last + 1):
                        if jb + 2 <= jb_last:
                            pend.append(emit_s(h, jb + 2))
                        emit_pv(h, jb, *pend.pop(0))
                else:
                    pend = [emit_s(h, 0) for h in heads]
                    for jb in range(jb_last + 1):
                        nxt = None
                        if jb < jb_last:
                            nxt = [emit_s(h, jb + 1) for h in heads]
                        for hi, h in enumerate(heads):
                            emit_pv(h, jb, *pend[hi])
                        if nxt is not None:
                            pend = nxt

                # normalize: recip of sums row (PSUM p64 -> SBUF p0, bf16),
                # broadcast across partitions via a K=1 PE matmul with a ones
                # column (the DMA broadcast costs ~2.5us of critical path per
                # use on HW), then fold the PSUM->SBUF copy into the multiply.
                isl = slice(i_base, i_base + 1024)
                for h in heads:
                    po2, mo2 = h % 2, h // 2
                    o_sl = rep.oT[64 * po2:64 * po2 + 64, mo2, isl]
                    if probe == "bcdma":
                        rpf = rppool.tile([65, 1024], F32, tag="rpf", name="rpf")
                        nc.vector.reciprocal(rpf[64:65, :], pos[h][64:65, :])
                        bc = bcpool.tile([64, 1024], F32, tag="bc", name="bc")
                        nc.sync.dma_start(
                            bc[:],
                            rpf[64:65, None, :].to_broadcast([1, 64, 1024])
                        )
                        nc.vector.tensor_tensor(
                            o_sl, pos[h][0:64, :], bc[:], MULT,
                        )
                        continue
                    rp = rppool.tile([1, 1024], BF16, tag="rp", name="rp")
                    nc.vector.reciprocal(rp[0:1, :], pos[h][64:65, :])
                    bc_ps = psum.tile([128, 1024], F32, tag="s",
                                      name="bc_ps", **s_kw)
                    for half in range(2):
                        hs = slice(half * 512, (half + 1) * 512)
                        nc.tensor.matmul(
                            bc_ps[0:64, hs], ones_sb[:], rp[0:1, hs],
                            start=True, stop=True,
                        )
                    nc.vector.tensor_copy(o_sl, pos[h][0:64, :])
                    nc.vector.tensor_tensor(o_sl, o_sl, bc_ps[0:64, :], MULT)

            def outproj_block(r, ic):
                """Output projection for one 1024-token half."""
                rep = get_rep(r)
                for tb2 in range(4 * ic, 4 * ic + 4):
                    y2 = ypool.tile([128, 2, C], BF16, tag="yt", name="y2")
                    for sub in range(2):
                        tb = 2 * tb2 + sub
                        for nk in range(2):
                            py = psum.tile([128, 1024], F32, tag="s",
                                           name="py", **s_kw)[:, 0:512]
                            for cp in range(2):
                                nc.tensor.matmul(
                                    py[:],
                                    rep.oT[:, cp, tb * 128:(tb + 1) * 128],
                                    wo_sb[:, cp, nk * 512:(nk + 1) * 512],
                                    start=(cp == 0),
                                    stop=(cp == 1),
                                )
                            nc.vector.tensor_copy(
                                y2[:, sub, nk * 512:(nk + 1) * 512], py[:])
                    nc.gpsimd.dma_start(
                        y[tb2 * 256:(tb2 + 1) * 256, :].rearrange(
                            "(b p) c -> p b c", p=128
                        ),
                        y2[:],
                    )
                    if probe == "2y":
                        nc.sync.dma_start(
                            y[tb2 * 256:(tb2 + 1) * 256, :].rearrange(
                                "(b p) c -> p b c", p=128
                            ),
                            y2[:],
                        )

            # ---------------- emission ----------------
            emit_xdma(0)
            phase1_block(0, t8=0)
            for r in range(nrep):
                if r + 1 < nrep:
                    emit_xdma(r + 1)
                groups = ([(h,) for h in range(NH)] if single3
                          else [(0, 1), (2, 3)])
                for g in groups:
                    att_group(r, g, ic=0)
                phase1_block(r, t8=1)   # ACT drains ic0 exp backlog here
                for g in groups:
                    att_group(r, g, ic=1)
                outproj_block(r, ic=0)
                outproj_block(r, ic=1)  # ACT drains ic1 exp backlog here
                if r + 1 < nrep:
                    phase1_block(r + 1, t8=0)

    nc.compile()
    return nc


def _get_nc():
    global _compiled
    if _compiled is None:
        _compiled = _build()
    return _compiled


class _Runner:
    """Compiled PJRT executor for the SPMD kernel, reusable across calls."""

    def __init__(self, nc):
        import jax
        import concourse.mybir as mybir
        from concourse import bass2jax
        from jax.experimental.shard_map import shard_map
        from jax.sharding import Mesh, PartitionSpec

        self.jax = jax
        self.nc = nc
        bass2jax.install_neuronx_cc_hook()

        partition_name = (
            nc.partition_id_tensor.name if nc.partition_id_tensor else None
        )
        in_names, out_names, out_avals, zero_outs = [], [], [], []
        for alloc in nc.m.functions[0].allocations:
            if not isinstance(alloc, mybir.MemoryLocationSet):
                continue
            name = alloc.memorylocations[0].name
            if alloc.kind == "ExternalInput":
                if name != partition_name:
                    in_names.append(name)
            elif alloc.kind == "ExternalOutput":
                out_names.append(name)
                shape = tuple(alloc.tensor_shape)
                dtype = mybir.dt.np(alloc.dtype)
                out_avals.append(jax.core.ShapedArray(shape, dtype))
                zero_outs.append(np.zeros(shape, dtype))
        self.in_names = in_names
        self.out_names = out_names
        self.out_avals = out_avals
        self.zero_outs = zero_outs
        all_names = tuple(in_names + out_names)

        if partition_name is not None:
            all_names = all_names + (partition_name,)

        def _body(*args):
            operands = list(args)
            if partition_name is not None:
                operands.append(bass2jax.partition_id_tensor())
            outs = bass2jax._bass_exec_p.bind(
                *operands,
                out_avals=tuple(out_avals),
                in_names=all_names,
                out_names=tuple(out_names),
                lowering_input_output_aliases=(),
                sim_require_finite=True,
                sim_require_nnan=True,
                nc=nc,
            )
            return tuple(outs)

        devices = jax.devices()[:NCORES]
        assert len(devices) == NCORES
        mesh = Mesh(np.asarray(devices), ("core",))
        self._sharding = jax.sharding.NamedSharding(mesh, PartitionSpec("core"))
        n_args = len(in_names) + len(out_names)
        self.fn = jax.jit(
            shard_map(
                _body,
                mesh=mesh,
                in_specs=(PartitionSpec("core"),) * n_args,
                out_specs=(PartitionSpec("core"),) * len(out_names),
                check_rep=False,
            ),
            keep_unused=True,
        )

    def device_args(self, in_maps):
        args = [
            np.concatenate([np.asarray(m[name]) for m in in_maps], axis=0)
            for name in self.in_names
        ]
        args += [
            np.zeros((NCORES * z.shape[0], *z.shape[1:]), z.dtype)
            for z in self.zero_outs
        ]
        return [self.jax.device_put(a, self._sharding) for a in args]

    def run_device(self, dev_args):
        return self.fn(*dev_args)

    def run(self, in_maps):
        out_arrs = self.fn(*self.device_args(in_maps))
        return [
            {
                name: np.asarray(out_arrs[i]).reshape(
                    NCORES, *self.out_avals[i].shape
                )[c]
                for i, name in enumerate(self.out_names)
            }
            for c in range(NCORES)
        ]


_runner = None


def _get_runner():
    global _runner
    if _runner is None:
        _runner = _Runner(_get_nc())
    return _runner


def make_in_maps(x, Wqkv, Wo):
    import ml_dtypes

    bf16 = ml_dtypes.bfloat16
    x = np.asarray(x, dtype=np.float32)
    Wqkv = np.asarray(Wqkv, dtype=np.float32)
    Wo = np.asarray(Wo, dtype=np.float32)
    mask = np.triu(np.ones((128, 128), dtype=np.float32)).astype(bf16)
    in_maps = []
    for c in range(NCORES):
        b, g = c // 4, c % 4
        in_maps.append({
            "xT": np.ascontiguousarray(x[b].T).astype(bf16),
            "wq": np.ascontiguousarray(
                Wqkv[:, g * CO:(g + 1) * CO]).astype(bf16),
            "wk": np.ascontiguousarray(
                Wqkv[:, C + g * CO:C + (g + 1) * CO]).astype(bf16),
            "wv": np.ascontiguousarray(
                Wqkv[:, 2 * C + g * CO:2 * C + (g + 1) * CO]).astype(bf16),
            "wo": np.ascontiguousarray(Wo[g * CO:(g + 1) * CO, :]).astype(bf16),
            "mask": mask,
        })
    return in_maps


def gather_output(results):
    y = np.zeros((B, T, C), dtype=np.float32)
    for c in range(NCORES):
        y[c // 4] += results[c]["y"].astype(np.float32)
    return y


def kernel(x, Wqkv, Wo):
    runner = _get_runner()
    in_maps = make_in_maps(x, Wqkv, Wo)
    return gather_output(runner.run(in_maps))


# revision 32
# speedup vs baseline: 1.1959x; 1.0428x over previous
"""Causal self-attention (B=2, T=2048, d_model=1024, H=16) on 8 TRN2 NeuronCores.

Sharding: core c handles batch b = c//4 and head group g = c%4 (heads 4g..4g+3).
Each core computes QKV projection for its heads, causal attention, and a partial
output projection y_partial = attn_out @ Wo[g*256:(g+1)*256, :]. The host sums
the 4 partials per batch (the tensor-parallel all-reduce, done on host) after
upcasting the bf16 partials.

Schedule per rep: att(ic=0) for both head pairs -> QKV projection of the second
token half -> att(ic=1) -> output projection -> next rep's first-half QKV.
Attention interleaves the two heads of a pair at j-block granularity so PE runs
~2 pipeline steps ahead of each exp, hiding the ACT handoff latency; the
projection blocks between attention sections give ACT time to drain its exp
backlog (they have no ACT work - all PSUM->SBUF copies are on DVE).

All matmul operands are bf16 (host converts); accumulation stays fp32 in PSUM.
PSUM: tag "s" 2x[128,1024] shared by S-tiles/projections, tag "o" 2 slots for
PV accumulators (one per in-flight head) shared with the V-projection.
"""
import sys

sys.path.insert(0, "/opt/trn_rl_repo")

import numpy as np

B, T, C = 2, 2048, 1024
NH_TOT = 16
HD = 64
NH = 4          # heads per core
CO = NH * HD    # 256 channels per core
NCORES = 8
SCALE = 1.0 / 32.0  # d_model ** -0.5

# attention schedule: "pair" = two heads interleaved, 2 S-slots;
# "single3" = one head per window, 3 S-slots (deeper S->exp WAR runahead)
ATT_MODE = "pair"

_compiled = None


def _build(nrep=1, trace_sim=False, att_mode=None, probe=None):
    import concourse.bass as bass  # noqa: F401
    import concourse.mybir as mybir
    import concourse.tile as tile
    from concourse import bacc

    F32 = mybir.dt.float32
    BF16 = mybir.dt.bfloat16
    MULT = mybir.AluOpType.mult
    EXP = mybir.ActivationFunctionType.Exp

    if att_mode is None:
        att_mode = ATT_MODE
    single3 = att_mode == "single3"
    s_kw = {"bufs": 3} if single3 else {}

    nc = bacc.Bacc("TRN2", target_bir_lowering=False)

    xT = nc.declare_dram_parameter("xT", [C, T], BF16, isOutput=False)
    wq = nc.declare_dram_parameter("wq", [C, CO], BF16, isOutput=False)
    wk = nc.declare_dram_parameter("wk", [C, CO], BF16, isOutput=False)
    wv = nc.declare_dram_parameter("wv", [C, CO], BF16, isOutput=False)
    wo = nc.declare_dram_parameter("wo", [CO, C], BF16, isOutput=False)
    mask = nc.declare_dram_parameter("mask", [128, 128], BF16, isOutput=False)
    y = nc.declare_dram_parameter("y", [T, C], BF16, isOutput=True)

    xT_t = xT.rearrange("(o p) t -> p o t", p=128)   # [128, 8, 2048]
    wq_t = wq.rearrange("(o p) m -> p o m", p=128)   # [128, 8, 256]
    wk_t = wk.rearrange("(o p) m -> p o m", p=128)
    wv_t = wv.rearrange("(o p) m -> p o m", p=128)
    wo_t = wo.rearrange("(o p) m -> p o m", p=128)   # [128, 2, 1024]

    with tile.TileContext(nc, trace_sim=trace_sim) as tc:
        with (
            nc.allow_low_precision(reason="bf16 matmul pipeline"),
            tc.tile_pool(name="wpool", bufs=1) as wpool,
            tc.tile_pool(name="xpool", bufs=2) as xpool,
            tc.tile_pool(name="qkvpool", bufs=2) as qkvpool,
            tc.tile_pool(name="otpool", bufs=2) as otpool,
            tc.tile_pool(name="etpool", bufs=6) as etpool,
            tc.tile_pool(name="rppool", bufs=2) as rppool,
            tc.tile_pool(name="bcpool", bufs=2) as bcpool,
            tc.tile_pool(name="ypool", bufs=2) as ypool,
            tc.tile_pool(name="psum", bufs=2, space="PSUM") as psum,
        ):
            wq_sb = wpool.tile([128, 8, CO], BF16, tag="wq")
            wk_sb = wpool.tile([128, 8, CO], BF16, tag="wk")
            wv_sb = wpool.tile([128, 8, CO], BF16, tag="wv")
            wo_sb = wpool.tile([128, 2, C], BF16, tag="wo")
            mask_sb = wpool.tile([128, 128], BF16, tag="mask")
            ones_sb = wpool.tile([1, 64], BF16, tag="ones")
            nc.vector.memset(ones_sb[:], 1.0)
            nc.sync.dma_start(wq_sb[:], wq_t[:])
            nc.sync.dma_start(wk_sb[:], wk_t[:])
            nc.sync.dma_start(wv_sb[:], wv_t[:])
            nc.sync.dma_start(wo_sb[:], wo_t[:])
            nc.sync.dma_start(mask_sb[:], mask[:])

            reps = {}

            class Rep:
                def __init__(self, r):
                    self.r = r
                    self.x = xpool.tile([128, 8, T], BF16, tag="xT", name=f"x{r}")
                    self.qT = qkvpool.tile([128, 2, T], BF16, tag="qT", name=f"q{r}")
                    self.kT = qkvpool.tile([128, 2, T], BF16, tag="kT", name=f"k{r}")
                    # V' per (t-block, head): 64 cols of V then a ones column
                    self.vp = qkvpool.tile([128, 16, NH, HD + 1], BF16, tag="vp",
                                           name=f"v{r}")
                    self.oT = otpool.tile([128, 2, T], BF16, tag="oT", name=f"o{r}")
                    nc.vector.memset(self.vp[:, :, :, HD], 1.0)

            def get_rep(r):
                if r not in reps:
                    reps[r] = Rep(r)
                return reps[r]

            def emit_xdma(r):
                rep = get_rep(r)
                for t8 in range(2):
                    for kc in range(8):
                        sl = slice(t8 * 1024, (t8 + 1) * 1024)
                        nc.sync.dma_start(rep.x[:, kc, sl], xT_t[:, kc, sl])

            def phase1_block(r, t8, part=None):
                """QKV projection for one 1024-token half (solid PE block).

                part 0/1 emit half each (interleaved between attention
                windows as ACT-catchup blocks); None emits everything.
                """
                rep = get_rep(r)
                ms = range(2) if part is None else [part]
                for m in ms:
                    for w_sb, dst in ((wk_sb, rep.kT), (wq_sb, rep.qT)):
                        for half in range(2):
                            pq = psum.tile([128, 1024], F32, tag="s",
                                           name="pq", **s_kw)[:, 0:512]
                            t0c = t8 * 1024 + half * 512
                            for kc in range(8):
                                nc.tensor.matmul(
                                    pq[:],
                                    w_sb[:, kc, m * 128:(m + 1) * 128],
                                    rep.x[:, kc, t0c:t0c + 512],
                                    start=(kc == 0),
                                    stop=(kc == 7),
                                )
                            nc.vector.tensor_copy(dst[:, m, t0c:t0c + 512], pq[:])
                if part is None:
                    tbs = range(8 * t8, 8 * t8 + 8)
                else:
                    tbs = range(8 * t8 + 4 * part, 8 * t8 + 4 * part + 4)
                for tb in tbs:
                    pv = psum.tile([128, 1024], F32, tag="s" if single3 else "o",
                                   name="pv", **s_kw)[:, 0:CO]
                    for kc in range(8):
                        nc.tensor.matmul(
                            pv[:],
                            rep.x[:, kc, tb * 128:(tb + 1) * 128],
                            wv_sb[:, kc, :],
                            start=(kc == 0),
                            stop=(kc == 7),
                        )
                    nc.vector.tensor_copy(
                        rep.vp[:, tb, :, 0:HD],
                        pv[:].rearrange("p (h d) -> p h d", h=NH),
                    )

            def att_group(r, heads, ic):
                """Causal attention for `heads`, query block ic (1024 wide).

                pair mode: two heads interleaved at j-block granularity.
                single3 mode: one head, S emitted two j-blocks ahead through
                three S-slots, widening the S->exp WAR runahead.
                """
                rep = get_rep(r)
                i_base = 1024 * ic
                jb_last = 8 * ic + 7
                pos = {
                    h: psum.tile([65, 1024], F32, tag="o",
                                 name=f"po{r}_{h}_{ic}",
                                 **({"bufs": 1} if single3 else {}))
                    for h in heads
                }

                def emit_s(h, jb):
                    po2, mo2 = h % 2, h // 2
                    k_h = rep.kT[64 * po2:64 * po2 + 64, mo2, :]
                    q_h = rep.qT[64 * po2:64 * po2 + 64, mo2, :]
                    i0 = max(i_base, 128 * jb)
                    ps_s = psum.tile([128, 1024], F32, tag="s", name="ps_s",
                                     **s_kw)
                    off = i0 - i_base
                    while off < 1024:
                        w = min(512 - off % 512, 1024 - off)
                        nc.tensor.matmul(
                            ps_s[:, off:off + w],
                            k_h[:, jb * 128:(jb + 1) * 128],
                            q_h[:, i_base + off:i_base + off + w],
                            start=True,
                            stop=True,
                        )
                        off += w
                    et = etpool.tile([128, 1024], BF16, tag="et", name="et")
                    o0 = i0 - i_base
                    nc.scalar.activation(
                        et[:, o0:1024], ps_s[:, o0:1024], EXP, scale=SCALE,
                    )
                    if probe == "2exp":
                        et2 = etpool.tile([128, 1024], BF16, tag="et2",
                                          name="et2")
                        nc.scalar.activation(
                            et2[:, o0:1024], ps_s[:, o0:1024], EXP, scale=SCALE,
                        )
                    if 128 * jb >= i_base:
                        nc.vector.tensor_tensor(
                            et[:, o0:o0 + 128], et[:, o0:o0 + 128],
                            mask_sb[:], MULT,
                        )
                    return et, i0

                def emit_pv(h, jb, et, i0):
                    # PSUM accumulation groups are bank-granular (2KB): close
                    # each bank's group on the last jb whose causal range
                    # still touches that bank.  The diagonal 128-strip depends
                    # on the DVE mask multiply - emit it LAST so the wide
                    # chunks (which only need the exp) keep PE busy while the
                    # mask handoff completes.
                    o0 = i0 - i_base
                    diag = 128 * jb >= i_base
                    chunks = []
                    off = o0 + 128 if diag else o0
                    while off < 1024:
                        w = min(512 - off % 512, 1024 - off)
                        chunks.append((off, w))
                        off += w
                    if diag:
                        chunks.append((o0, min(128, 1024 - o0)))
                    first_idx, last_idx = {}, {}
                    for idx, (off, w) in enumerate(chunks):
                        bank = off // 512
                        if bank not in first_idx:
                            first_idx[bank] = idx
                        last_idx[bank] = idx
                    for idx, (off, w) in enumerate(chunks):
                        bank = off // 512
                        jb_stop = min(jb_last,
                                      (i_base + 512 * (bank + 1) - 1) // 128)
                        nc.tensor.matmul(
                            pos[h][:, off:off + w],
                            rep.vp[:, jb, h, :],
                            et[:, off:off + w],
                            start=(jb == 0 and first_idx[bank] == idx),
                            stop=(jb == jb_stop and last_idx[bank] == idx),
                        )

                if single3:
                    h = heads[0]
                    pend = [emit_s(h, 0)]
                    if jb_last >= 1:
                        pend.append(emit_s(h, 1))
                    for jb in range(jb_last + 1):
                        if jb + 2 <= jb_last:
                            pend.append(emit_s(h, jb + 2))
                        emit_pv(h, jb, *pend.pop(0))
                else:
                    pend = [emit_s(h, 0) for h in heads]
                    for jb in range(jb_last + 1):
                        nxt = None
                        if jb < jb_last:
                            nxt = [emit_s(h, jb + 1) for h in heads]
                        for hi, h in enumerate(heads):
                            emit_pv(h, jb, *pend[hi])
                        if nxt is not None:
                            pend = nxt

                # normalize: recip of sums row (PSUM p64 -> SBUF p0, bf16),
                # broadcast across partitions via a K=1 PE matmul with a ones
                # column (the DMA broadcast costs ~2.5us of critical path per
                # use on HW), then fold the PSUM->SBUF copy into the multiply.
                isl = slice(i_base, i_base + 1024)
                for h in heads:
                    po2, mo2 = h % 2, h // 2
                    o_sl = rep.oT[64 * po2:64 * po2 + 64, mo2, isl]
                    if probe == "bcdma":
                        rpf = rppool.tile([65, 1024], F32, tag="rpf", name="rpf")
                        nc.vector.reciprocal(rpf[64:65, :], pos[h][64:65, :])
                        bc = bcpool.tile([64, 1024], F32, tag="bc", name="bc")
                        nc.sync.dma_start(
                            bc[:],
                            rpf[64:65, None, :].to_broadcast([1, 64, 1024])
                        )
                        nc.vector.tensor_tensor(
                            o_sl, pos[h][0:64, :], bc[:], MULT,
                        )
                        continue
                    rp = rppool.tile([1, 1024], BF16, tag="rp", name="rp")
                    nc.vector.reciprocal(rp[0:1, :], pos[h][64:65, :])
                    bc_ps = psum.tile([128, 1024], F32, tag="s",
                                      name="bc_ps", **s_kw)
                    for half in range(2):
                        hs = slice(half * 512, (half + 1) * 512)
                        nc.tensor.matmul(
                            bc_ps[0:64, hs], ones_sb[:], rp[0:1, hs],
                            start=True, stop=True,
                        )
                    nc.vector.tensor_copy(o_sl, pos[h][0:64, :])
                    nc.vector.tensor_tensor(o_sl, o_sl, bc_ps[0:64, :], MULT)

            def outproj_block(r, ic):
                """Output projection for one 1024-token half."""
                rep = get_rep(r)
                for tb2 in range(4 * ic, 4 * ic + 4):
                    y2 = ypool.tile([128, 2, C], BF16, tag="yt", name="y2")
                    for sub in range(2):
                        tb = 2 * tb2 + sub
                        for nk in range(2):
                            py = psum.tile([128, 1024], F32, tag="s",
                                           name="py", **s_kw)[:, 0:512]
                            for cp in range(2):
                                nc.tensor.matmul(
                                    py[:],
                                    rep.oT[:, cp, tb * 128:(tb + 1) * 128],
                                    wo_sb[:, cp, nk * 512:(nk + 1) * 512],
                                    start=(cp == 0),
                                    stop=(cp == 1),
                                )
                            nc.vector.tensor_copy(
                                y2[:, sub, nk * 512:(nk + 1) * 512], py[:])
                    nc.gpsimd.dma_start(
                        y[tb2 * 256:(tb2 + 1) * 256, :].rearrange(
                            "(b p) c -> p b c", p=128
                        ),
                        y2[:],
                    )
                    if probe == "2y":
                        nc.sync.dma_start(
                            y[tb2 * 256:(tb2 + 1) * 256, :].rearrange(
                                "(b p) c -> p b c", p=128
                            ),
                            y2[:],
                        )

            # ---------------- emission ----------------
            emit_xdma(0)
            phase1_block(0, t8=0)
            for r in range(nrep):
                if r + 1 < nrep:
                    emit_xdma(r + 1)
                if single3:
                    for h in range(NH):
                        att_group(r, (h,), ic=0)
                    phase1_block(r, t8=1)
                    for h in range(NH):
                        att_group(r, (h,), ic=1)
                    outproj_block(r, ic=0)
                    outproj_block(r, ic=1)
                    if r + 1 < nrep:
                        phase1_block(r + 1, t8=0)
                elif probe == "coarse":
                    att_group(r, (0, 1), ic=0)
                    att_group(r, (2, 3), ic=0)
                    phase1_block(r, t8=1)
                    att_group(r, (0, 1), ic=1)
                    att_group(r, (2, 3), ic=1)
                    outproj_block(r, ic=0)
                    outproj_block(r, ic=1)
                    if r + 1 < nrep:
                        phase1_block(r + 1, t8=0)
                else:
                    # a PE-only catchup block follows EVERY attention window,
                    # sized to that window's ACT exp deficit, so no window
                    # starts with accumulated exp backlog.
                    att_group(r, (0, 1), ic=0)
                    phase1_block(r, t8=1, part=0)
                    att_group(r, (2, 3), ic=0)
                    phase1_block(r, t8=1, part=1)
                    att_group(r, (0, 1), ic=1)
                    outproj_block(r, ic=0)
                    att_group(r, (2, 3), ic=1)
                    outproj_block(r, ic=1)
                    if r + 1 < nrep:
                        phase1_block(r + 1, t8=0)

    nc.compile()
    return nc


def _get_nc():
    global _compiled
    if _compiled is None:
        _compiled = _build()
    return _compiled


class _Runner:
    """Compiled PJRT executor for the SPMD kernel, reusable across calls."""

    def __init__(self, nc):
        import jax
        import concourse.mybir as mybir
        from concourse import bass2jax
        from jax.experimental.shard_map import shard_map
        from jax.sharding import Mesh, PartitionSpec

        self.jax = jax
        self.nc = nc
        bass2jax.install_neuronx_cc_hook()

        partition_name = (
            nc.partition_id_tensor.name if nc.partition_id_tensor else None
        )
        in_names, out_names, out_avals, zero_outs = [], [], [], []
        for alloc in nc.m.functions[0].allocations:
            if not isinstance(alloc, mybir.MemoryLocationSet):
                continue
            name = alloc.memorylocations[0].name
            if alloc.kind == "ExternalInput":
                if name != partition_name:
                    in_names.append(name)
            elif alloc.kind == "ExternalOutput":
                out_names.append(name)
                shape = tuple(alloc.tensor_shape)
                dtype = mybir.dt.np(alloc.dtype)
                out_avals.append(jax.core.ShapedArray(shape, dtype))
                zero_outs.append(np.zeros(shape, dtype))
        self.in_names = in_names
        self.out_names = out_names
        self.out_avals = out_avals
        self.zero_outs = zero_outs
        all_names = tuple(in_names + out_names)

        if partition_name is not None:
            all_names = all_names + (partition_name,)

        def _body(*args):
            operands = list(args)
            if partition_name is not None:
                operands.append(bass2jax.partition_id_tensor())
            outs = bass2jax._bass_exec_p.bind(
                *operands,
                out_avals=tuple(out_avals),
                in_names=all_names,
                out_names=tuple(out_names),
                lowering_input_output_aliases=(),
                sim_require_finite=True,
                sim_require_nnan=True,
                nc=nc,
            )
            return tuple(outs)

        devices = jax.devices()[:NCORES]
        assert len(devices) == NCORES
        mesh = Mesh(np.asarray(devices), ("core",))
        self._sharding = jax.sharding.NamedSharding(mesh, PartitionSpec("core"))
        n_args = len(in_names) + len(out_names)
        self.fn = jax.jit(
            shard_map(
                _body,
                mesh=mesh,
                in_specs=(PartitionSpec("core"),) * n_args,
                out_specs=(PartitionSpec("core"),) * len(out_names),
                check_rep=False,
            ),
            keep_unused=True,
        )

    def device_args(self, in_maps):
        args = [
            np.concatenate([np.asarray(m[name]) for m in in_maps], axis=0)
            for name in self.in_names
        ]
        args += [
            np.zeros((NCORES * z.shape[0], *z.shape[1:]), z.dtype)
            for z in self.zero_outs
        ]
        return [self.jax.device_put(a, self._sharding) for a in args]

    def run_device(self, dev_args):
        return self.fn(*dev_args)

    def run(self, in_maps):
        out_arrs = self.fn(*self.device_args(in_maps))
        return [
            {
                name: np.asarray(out_arrs[i]).reshape(
                    NCORES, *self.out_avals[i].shape
                )[c]
                for i, name in enumerate(self.out_names)
            }
            for c in range(NCORES)
        ]


_runner = None


def _get_runner():
    global _runner
    if _runner is None:
        _runner = _Runner(_get_nc())
    return _runner


def make_in_maps(x, Wqkv, Wo):
    import ml_dtypes

    bf16 = ml_dtypes.bfloat16
    x = np.asarray(x, dtype=np.float32)
    Wqkv = np.asarray(Wqkv, dtype=np.float32)
    Wo = np.asarray(Wo, dtype=np.float32)
    mask = np.triu(np.ones((128, 128), dtype=np.float32)).astype(bf16)
    in_maps = []
    for c in range(NCORES):
        b, g = c // 4, c % 4
        in_maps.append({
            "xT": np.ascontiguousarray(x[b].T).astype(bf16),
            "wq": np.ascontiguousarray(
                Wqkv[:, g * CO:(g + 1) * CO]).astype(bf16),
            "wk": np.ascontiguousarray(
                Wqkv[:, C + g * CO:C + (g + 1) * CO]).astype(bf16),
            "wv": np.ascontiguousarray(
                Wqkv[:, 2 * C + g * CO:2 * C + (g + 1) * CO]).astype(bf16),
            "wo": np.ascontiguousarray(Wo[g * CO:(g + 1) * CO, :]).astype(bf16),
            "mask": mask,
        })
    return in_maps


def gather_output(results):
    y = np.zeros((B, T, C), dtype=np.float32)
    for c in range(NCORES):
        y[c // 4] += results[c]["y"].astype(np.float32)
    return y


def kernel(x, Wqkv, Wo):
    runner = _get_runner()
    in_maps = make_in_maps(x, Wqkv, Wo)
    return gather_output(runner.run(in_maps))


# revision 35
# speedup vs baseline: 1.3812x; 1.1550x over previous
"""Causal self-attention (B=2, T=2048, d_model=1024, H=16) on 8 TRN2 NeuronCores.

Sharding: core c handles batch b = c//4 and head group g = c%4 (heads 4g..4g+3).
Each core computes QKV projection for its heads, causal attention, and a partial
output projection y_partial = attn_out @ Wo[g*256:(g+1)*256, :]. The host sums
the 4 partials per batch (the tensor-parallel all-reduce, done on host) after
upcasting the bf16 partials.

Schedule per rep: att(ic=0) for both head pairs -> QKV projection of the second
token half -> att(ic=1) -> output projection -> next rep's first-half QKV.
Attention interleaves the two heads of a pair at j-block granularity so PE runs
~2 pipeline steps ahead of each exp, hiding the ACT handoff latency; the
projection blocks between attention sections give ACT time to drain its exp
backlog (they have no ACT work - all PSUM->SBUF copies are on DVE).

All matmul operands are bf16 (host converts); accumulation stays fp32 in PSUM.
PSUM: tag "s" 2x[128,1024] shared by S-tiles/projections, tag "o" 2 slots for
PV accumulators (one per in-flight head) shared with the V-projection.
"""
import sys

sys.path.insert(0, "/opt/trn_rl_repo")

import numpy as np

B, T, C = 2, 2048, 1024
NH_TOT = 16
HD = 64
NH = 4          # heads per core
CO = NH * HD    # 256 channels per core
NCORES = 8
SCALE = 1.0 / 32.0  # d_model ** -0.5

# attention schedule: "pair" = two heads interleaved, 2 S-slots;
# "single3" = one head per window, 3 S-slots (deeper S->exp WAR runahead)
ATT_MODE = "pair"

_compiled = None


def _build(nrep=1, trace_sim=False, att_mode=None, probe=None):
    import concourse.bass as bass  # noqa: F401
    import concourse.mybir as mybir
    import concourse.tile as tile
    from concourse import bacc

    F32 = mybir.dt.float32
    BF16 = mybir.dt.bfloat16
    MULT = mybir.AluOpType.mult
    EXP = mybir.ActivationFunctionType.Exp

    if att_mode is None:
        att_mode = ATT_MODE
    single3 = att_mode == "single3"
    s_kw = {"bufs": 3} if single3 else {}

    nc = bacc.Bacc("TRN2", target_bir_lowering=False)

    xT = nc.declare_dram_parameter("xT", [C, T], BF16, isOutput=False)
    wq = nc.declare_dram_parameter("wq", [C, CO], BF16, isOutput=False)
    wk = nc.declare_dram_parameter("wk", [C, CO], BF16, isOutput=False)
    wv = nc.declare_dram_parameter("wv", [C, CO], BF16, isOutput=False)
    wo = nc.declare_dram_parameter("wo", [CO, C], BF16, isOutput=False)
    mask = nc.declare_dram_parameter("mask", [128, 128], BF16, isOutput=False)
    y = nc.declare_dram_parameter("y", [T, C], BF16, isOutput=True)

    xT_t = xT.rearrange("(o p) t -> p o t", p=128)   # [128, 8, 2048]
    wq_t = wq.rearrange("(o p) m -> p o m", p=128)   # [128, 8, 256]
    wk_t = wk.rearrange("(o p) m -> p o m", p=128)
    wv_t = wv.rearrange("(o p) m -> p o m", p=128)
    wo_t = wo.rearrange("(o p) m -> p o m", p=128)   # [128, 2, 1024]

    with tile.TileContext(nc, trace_sim=trace_sim) as tc:
        with (
            nc.allow_low_precision(reason="bf16 matmul pipeline"),
            tc.tile_pool(name="wpool", bufs=1) as wpool,
            tc.tile_pool(name="xpool", bufs=2) as xpool,
            tc.tile_pool(name="qkvpool", bufs=2) as qkvpool,
            tc.tile_pool(name="otpool", bufs=2) as otpool,
            tc.tile_pool(name="etpool", bufs=6) as etpool,
            tc.tile_pool(name="rppool", bufs=2) as rppool,
            tc.tile_pool(name="bcpool", bufs=2) as bcpool,
            tc.tile_pool(name="ypool", bufs=3) as ypool,
            tc.tile_pool(name="psum", bufs=2, space="PSUM") as psum,
        ):
            wq_sb = wpool.tile([128, 8, CO], BF16, tag="wq")
            wk_sb = wpool.tile([128, 8, CO], BF16, tag="wk")
            wv_sb = wpool.tile([128, 8, CO], BF16, tag="wv")
            wo_sb = wpool.tile([128, 2, C], BF16, tag="wo")
            mask_sb = wpool.tile([128, 128], BF16, tag="mask")
            ones_sb = wpool.tile([1, 64], BF16, tag="ones")
            nc.vector.memset(ones_sb[:], 1.0)
            nc.sync.dma_start(wq_sb[:], wq_t[:])
            nc.sync.dma_start(wk_sb[:], wk_t[:])
            nc.sync.dma_start(wv_sb[:], wv_t[:])
            nc.sync.dma_start(wo_sb[:], wo_t[:])
            nc.sync.dma_start(mask_sb[:], mask[:])

            reps = {}

            class Rep:
                def __init__(self, r):
                    self.r = r
                    self.x = xpool.tile([128, 8, T], BF16, tag="xT", name=f"x{r}")
                    self.qT = qkvpool.tile([128, 2, T], BF16, tag="qT", name=f"q{r}")
                    self.kT = qkvpool.tile([128, 2, T], BF16, tag="kT", name=f"k{r}")
                    # V' per (t-block, head): 64 cols of V then a ones column
                    self.vp = qkvpool.tile([128, 16, NH, HD + 1], BF16, tag="vp",
                                           name=f"v{r}")
                    self.oT = otpool.tile([128, 2, T], BF16, tag="oT", name=f"o{r}")
                    nc.vector.memset(self.vp[:, :, :, HD], 1.0)

            def get_rep(r):
                if r not in reps:
                    reps[r] = Rep(r)
                return reps[r]

            def emit_xdma(r):
                rep = get_rep(r)
                for t8 in range(2):
                    for kc in range(8):
                        sl = slice(t8 * 1024, (t8 + 1) * 1024)
                        nc.sync.dma_start(rep.x[:, kc, sl], xT_t[:, kc, sl])

            def phase1_block(r, t8, part=None):
                """QKV projection for one 1024-token half (solid PE block).

                part 0/1 emit half each (interleaved between attention
                windows as ACT-catchup blocks); None emits everything.
                """
                rep = get_rep(r)
                ms = range(2) if part is None else [part]
                for m in ms:
                    for w_sb, dst in ((wk_sb, rep.kT), (wq_sb, rep.qT)):
                        for half in range(2):
                            pq = psum.tile([128, 1024], F32, tag="s",
                                           name="pq", **s_kw)[:, 0:512]
                            t0c = t8 * 1024 + half * 512
                            for kc in range(8):
                                nc.tensor.matmul(
                                    pq[:],
                                    w_sb[:, kc, m * 128:(m + 1) * 128],
                                    rep.x[:, kc, t0c:t0c + 512],
                                    start=(kc == 0),
                                    stop=(kc == 7),
                                )
                            nc.vector.tensor_copy(dst[:, m, t0c:t0c + 512], pq[:])
                if part is None:
                    tbs = range(8 * t8, 8 * t8 + 8)
                else:
                    tbs = range(8 * t8 + 4 * part, 8 * t8 + 4 * part + 4)
                for tb in tbs:
                    pv = psum.tile([128, 1024], F32, tag="s" if single3 else "o",
                                   name="pv", **s_kw)[:, 0:CO]
                    for kc in range(8):
                        nc.tensor.matmul(
                            pv[:],
                            rep.x[:, kc, tb * 128:(tb + 1) * 128],
                            wv_sb[:, kc, :],
                            start=(kc == 0),
                            stop=(kc == 7),
                        )
                    nc.vector.tensor_copy(
                        rep.vp[:, tb, :, 0:HD],
                        pv[:].rearrange("p (h d) -> p h d", h=NH),
                    )

            def att_group(r, heads, ic):
                """Causal attention for `heads`, query block ic (1024 wide).

                pair mode: two heads interleaved at j-block granularity.
                single3 mode: one head, S emitted two j-blocks ahead through
                three S-slots, widening the S->exp WAR runahead.
                """
                rep = get_rep(r)
                i_base = 1024 * ic
                jb_last = 8 * ic + 7
                pos = {
                    h: psum.tile([65, 1024], F32, tag="o",
                                 name=f"po{r}_{h}_{ic}",
                                 **({"bufs": 1} if single3 else {}))
                    for h in heads
                }

                def emit_s(h, jb):
                    po2, mo2 = h % 2, h // 2
                    k_h = rep.kT[64 * po2:64 * po2 + 64, mo2, :]
                    q_h = rep.qT[64 * po2:64 * po2 + 64, mo2, :]
                    i0 = max(i_base, 128 * jb)
                    ps_s = psum.tile([128, 1024], F32, tag="s", name="ps_s",
                                     **s_kw)
                    off = i0 - i_base
                    while off < 1024:
                        w = min(512 - off % 512, 1024 - off)
                        nc.tensor.matmul(
                            ps_s[:, off:off + w],
                            k_h[:, jb * 128:(jb + 1) * 128],
                            q_h[:, i_base + off:i_base + off + w],
                            start=True,
                            stop=True,
                        )
                        off += w
                    et = etpool.tile([128, 1024], BF16, tag="et", name="et")
                    o0 = i0 - i_base
                    nc.scalar.activation(
                        et[:, o0:1024], ps_s[:, o0:1024], EXP, scale=SCALE,
                    )
                    if probe == "2exp":
                        et2 = etpool.tile([128, 1024], BF16, tag="et2",
                                          name="et2")
                        nc.scalar.activation(
                            et2[:, o0:1024], ps_s[:, o0:1024], EXP, scale=SCALE,
                        )
                    if 128 * jb >= i_base:
                        nc.vector.tensor_tensor(
                            et[:, o0:o0 + 128], et[:, o0:o0 + 128],
                            mask_sb[:], MULT,
                        )
                    return et, i0

                def emit_pv(h, jb, et, i0):
                    # PSUM accumulation groups are bank-granular (2KB): close
                    # each bank's group on the last jb whose causal range
                    # still touches that bank.  The diagonal 128-strip depends
                    # on the DVE mask multiply - emit it LAST so the wide
                    # chunks (which only need the exp) keep PE busy while the
                    # mask handoff completes.
                    o0 = i0 - i_base
                    diag = 128 * jb >= i_base
                    chunks = []
                    off = o0 + 128 if diag else o0
                    while off < 1024:
                        w = min(512 - off % 512, 1024 - off)
                        chunks.append((off, w))
                        off += w
                    if diag:
                        chunks.append((o0, min(128, 1024 - o0)))
                    first_idx, last_idx = {}, {}
                    for idx, (off, w) in enumerate(chunks):
                        bank = off // 512
                        if bank not in first_idx:
                            first_idx[bank] = idx
                        last_idx[bank] = idx
                    for idx, (off, w) in enumerate(chunks):
                        bank = off // 512
                        jb_stop = min(jb_last,
                                      (i_base + 512 * (bank + 1) - 1) // 128)
                        nc.tensor.matmul(
                            pos[h][:, off:off + w],
                            rep.vp[:, jb, h, :],
                            et[:, off:off + w],
                            start=(jb == 0 and first_idx[bank] == idx),
                            stop=(jb == jb_stop and last_idx[bank] == idx),
                        )

                if single3:
                    h = heads[0]
                    pend = [emit_s(h, 0)]
                    if jb_last >= 1:
                        pend.append(emit_s(h, 1))
                    for jb in range(jb_last + 1):
                        if jb + 2 <= jb_last:
                            pend.append(emit_s(h, jb + 2))
                        emit_pv(h, jb, *pend.pop(0))
                else:
                    pend = [emit_s(h, 0) for h in heads]
                    for jb in range(jb_last + 1):
                        nxt = None
                        if jb < jb_last:
                            nxt = [emit_s(h, jb + 1) for h in heads]
                        for hi, h in enumerate(heads):
                            emit_pv(h, jb, *pend[hi])
                        if nxt is not None:
                            pend = nxt

                # normalize: recip of sums row (PSUM p64 -> SBUF p0, bf16),
                # broadcast across partitions via a K=1 PE matmul with a ones
                # column (the DMA broadcast costs ~2.5us of critical path per
                # use on HW), then fold the PSUM->SBUF copy into the multiply.
                isl = slice(i_base, i_base + 1024)
                for h in heads:
                    po2, mo2 = h % 2, h // 2
                    o_sl = rep.oT[64 * po2:64 * po2 + 64, mo2, isl]
                    if probe == "bcdma":
                        rpf = rppool.tile([65, 1024], F32, tag="rpf", name="rpf")
                        nc.vector.reciprocal(rpf[64:65, :], pos[h][64:65, :])
                        bc = bcpool.tile([64, 1024], F32, tag="bc", name="bc")
                        nc.sync.dma_start(
                            bc[:],
                            rpf[64:65, None, :].to_broadcast([1, 64, 1024])
                        )
                        nc.vector.tensor_tensor(
                            o_sl, pos[h][0:64, :], bc[:], MULT,
                        )
                        continue
                    rp = rppool.tile([1, 1024], BF16, tag="rp", name="rp")
                    nc.vector.reciprocal(rp[0:1, :], pos[h][64:65, :])
                    # Drain O to SBUF, then overwrite the dead pos rows with
                    # the PE-broadcast reciprocal (start=False: no bank clear,
                    # so the copy-before-write WAR is the only ordering needed
                    # and no tag-s slot is consumed at the window boundary).
                    nc.vector.tensor_copy(o_sl, pos[h][0:64, :])
                    for half in range(2):
                        hs = slice(half * 512, (half + 1) * 512)
                        nc.tensor.matmul(
                            pos[h][0:64, hs], ones_sb[:], rp[0:1, hs],
                            start=True, stop=True, skip_group_check=True,
                        )
                    nc.vector.tensor_tensor(o_sl, o_sl, pos[h][0:64, :], MULT)

            def outproj_block(r, ic):
                """Output projection for one 1024-token half."""
                rep = get_rep(r)
                for tb2 in range(4 * ic, 4 * ic + 4):
                    y2 = ypool.tile([128, 2, C], BF16, tag="yt", name="y2",
                                    **({"bufs": 2} if probe == "ybuf2" else {}))
                    for sub in range(2):
                        tb = 2 * tb2 + sub
                        for nk in range(2):
                            py = psum.tile([128, 1024], F32, tag="s",
                                           name="py", **s_kw)[:, 0:512]
                            for cp in range(2):
                                nc.tensor.matmul(
                                    py[:],
                                    rep.oT[:, cp, tb * 128:(tb + 1) * 128],
                                    wo_sb[:, cp, nk * 512:(nk + 1) * 512],
                                    start=(cp == 0),
                                    stop=(cp == 1),
                                )
                            nc.vector.tensor_copy(
                                y2[:, sub, nk * 512:(nk + 1) * 512], py[:])
                    (nc.sync if probe == "ysp" else nc.gpsimd).dma_start(
                        y[tb2 * 256:(tb2 + 1) * 256, :].rearrange(
                            "(b p) c -> p b c", p=128
                        ),
                        y2[:],
                    )
                    if probe == "2y":
                        nc.sync.dma_start(
                            y[tb2 * 256:(tb2 + 1) * 256, :].rearrange(
                                "(b p) c -> p b c", p=128
                            ),
                            y2[:],
                        )

            # ---------------- emission ----------------
            emit_xdma(0)
            phase1_block(0, t8=0)
            for r in range(nrep):
                if r + 1 < nrep:
                    emit_xdma(r + 1)
                if single3:
                    for h in range(NH):
                        att_group(r, (h,), ic=0)
                    phase1_block(r, t8=1)
                    for h in range(NH):
                        att_group(r, (h,), ic=1)
                    outproj_block(r, ic=0)
                    outproj_block(r, ic=1)
                    if r + 1 < nrep:
                        phase1_block(r + 1, t8=0)
                elif probe == "coarse":
                    att_group(r, (0, 1), ic=0)
                    att_group(r, (2, 3), ic=0)
                    phase1_block(r, t8=1)
                    att_group(r, (0, 1), ic=1)
                    att_group(r, (2, 3), ic=1)
                    outproj_block(r, ic=0)
                    outproj_block(r, ic=1)
                    if r + 1 < nrep:
                        phase1_block(r + 1, t8=0)
                else:
                    # a PE-only catchup block follows EVERY attention window,
                    # sized to that window's ACT exp deficit, so no window
                    # starts with accumulated exp backlog.
                    att_group(r, (0, 1), ic=0)
                    phase1_block(r, t8=1, part=0)
                    att_group(r, (2, 3), ic=0)
                    phase1_block(r, t8=1, part=1)
                    att_group(r, (0, 1), ic=1)
                    outproj_block(r, ic=0)
                    att_group(r, (2, 3), ic=1)
                    outproj_block(r, ic=1)
                    if r + 1 < nrep:
                        phase1_block(r + 1, t8=0)

    nc.compile()
    return nc


def _get_nc():
    global _compiled
    if _compiled is None:
        _compiled = _build()
    return _compiled


class _Runner:
    """Compiled PJRT executor for the SPMD kernel, reusable across calls."""

    def __init__(self, nc):
        import jax
        import concourse.mybir as mybir
        from concourse import bass2jax
        from jax.experimental.shard_map import shard_map
        from jax.sharding import Mesh, PartitionSpec

        self.jax = jax
        self.nc = nc
        bass2jax.install_neuronx_cc_hook()

        partition_name = (
            nc.partition_id_tensor.name if nc.partition_id_tensor else None
        )
        in_names, out_names, out_avals, zero_outs = [], [], [], []
        for alloc in nc.m.functions[0].allocations:
            if not isinstance(alloc, mybir.MemoryLocationSet):
                continue
            name = alloc.memorylocations[0].name
            if alloc.kind == "ExternalInput":
                if name != partition_name:
                    in_names.append(name)
            elif alloc.kind == "ExternalOutput":
                out_names.append(name)
                shape = tuple(alloc.tensor_shape)
                dtype = mybir.dt.np(alloc.dtype)
                out_avals.append(jax.core.ShapedArray(shape, dtype))
                zero_outs.append(np.zeros(shape, dtype))
        self.in_names = in_names
        self.out_names = out_names
        self.out_avals = out_avals
        self.zero_outs = zero_outs
        all_names = tuple(in_names + out_names)

        if partition_name is not None:
            all_names = all_names + (partition_name,)

        def _body(*args):
            operands = list(args)
            if partition_name is not None:
                operands.append(bass2jax.partition_id_tensor())
            outs = bass2jax._bass_exec_p.bind(
                *operands,
                out_avals=tuple(out_avals),
                in_names=all_names,
                out_names=tuple(out_names),
                lowering_input_output_aliases=(),
                sim_require_finite=True,
                sim_require_nnan=True,
                nc=nc,
            )
            return tuple(outs)

        devices = jax.devices()[:NCORES]
        assert len(devices) == NCORES
        mesh = Mesh(np.asarray(devices), ("core",))
        self._sharding = jax.sharding.NamedSharding(mesh, PartitionSpec("core"))
        n_args = len(in_names) + len(out_names)
        self.fn = jax.jit(
            shard_map(
                _body,
                mesh=mesh,
                in_specs=(PartitionSpec("core"),) * n_args,
                out_specs=(PartitionSpec("core"),) * len(out_names),
                check_rep=False,
            ),
            keep_unused=True,
        )

    def device_args(self, in_maps):
        args = [
            np.concatenate([np.asarray(m[name]) for m in in_maps], axis=0)
            for name in self.in_names
        ]
        args += [
            np.zeros((NCORES * z.shape[0], *z.shape[1:]), z.dtype)
            for z in self.zero_outs
        ]
        return [self.jax.device_put(a, self._sharding) for a in args]

    def run_device(self, dev_args):
        return self.fn(*dev_args)

    def run(self, in_maps):
        out_arrs = self.fn(*self.device_args(in_maps))
        return [
            {
                name: np.asarray(out_arrs[i]).reshape(
                    NCORES, *self.out_avals[i].shape
                )[c]
                for i, name in enumerate(self.out_names)
            }
            for c in range(NCORES)
        ]


_runner = None


def _get_runner():
    global _runner
    if _runner is None:
        _runner = _Runner(_get_nc())
    return _runner


def make_in_maps(x, Wqkv, Wo):
    import ml_dtypes

    bf16 = ml_dtypes.bfloat16
    x = np.asarray(x, dtype=np.float32)
    Wqkv = np.asarray(Wqkv, dtype=np.float32)
    Wo = np.asarray(Wo, dtype=np.float32)
    mask = np.triu(np.ones((128, 128), dtype=np.float32)).astype(bf16)
    in_maps = []
    for c in range(NCORES):
        b, g = c // 4, c % 4
        in_maps.append({
            "xT": np.ascontiguousarray(x[b].T).astype(bf16),
            "wq": np.ascontiguousarray(
                Wqkv[:, g * CO:(g + 1) * CO]).astype(bf16),
            "wk": np.ascontiguousarray(
                Wqkv[:, C + g * CO:C + (g + 1) * CO]).astype(bf16),
            "wv": np.ascontiguousarray(
                Wqkv[:, 2 * C + g * CO:2 * C + (g + 1) * CO]).astype(bf16),
            "wo": np.ascontiguousarray(Wo[g * CO:(g + 1) * CO, :]).astype(bf16),
            "mask": mask,
        })
    return in_maps


def gather_output(results):
    y = np.zeros((B, T, C), dtype=np.float32)
    for c in range(NCORES):
        y[c // 4] += results[c]["y"].astype(np.float32)
    return y


def kernel(x, Wqkv, Wo):
    runner = _get_runner()
    in_maps = make_in_maps(x, Wqkv, Wo)
    return gather_output(runner.run(in_maps))
